# revision 22
# baseline (speedup 1.0000x reference)
"""Trainium2 Bass kernel for nn_NestedNarx: batched NARX MLP over basins.

Math (from the reference):
  For t >= 3:  xt(t) = 32 delayed features of x  ->  h = relu(W_in xt + b_in)
               a = tanh(W_ih h + b_ih + b_hh)
               y(t) = W_out a + b_out
  For t < 3:   y(t) = x[t, :, 7]

The 32 reference features (with a duplicated t-1 slice) fold into 24 distinct
features = all 8 channels at delays 1,2,3.  The host pre-builds the delay
stack in channel-major layout, so layer 1 is a single K=96 matmul per basin
pair with time on the free axis.  Basin pairs ride block-diagonal K=128
weights, keeping every matmul a plain position-(0,0) fp32r matmul:

  L1  psH[:, q] = l1[q].T @ xs[96, N]          (1 matmul per pair)
  L2  psA[:, q] = diag(W_ih^T, W_ih^T).T @ H   (1 matmul per pair)
  L3  psY[16, N] += l3[p].T @ A[:, q]          (1 matmul per pair, accum)

Sharding: pure data-parallel over the 1024 basins -> 8 cores x 128 basins.

Device tensors (per core):
  xp [32 strips, 96, 4096]  strip = 4 basins; row = 24*b + 8*(d-1) + c;
                            col v holds x[v-d, basin, c]  (zeros for v<d)
  y  [8 blocks, 16, 4096]   y[J, b_local, t]
"""

import os
import sys

import numpy as np

for _p in ("/opt/trn_rl_repo",):
    if _p not in sys.path and os.path.isdir(_p):
        sys.path.insert(0, _p)

import concourse.bass as bass
import concourse.mybir as mybir
from concourse.tile import TileContext

F32 = mybir.dt.float32
F32R = mybir.dt.float32r
AF = mybir.ActivationFunctionType

T = 4096
NG_ALL = 1024
NCORES = 8
G_CORE = NG_ALL // NCORES  # 128 basins per core
NSTRIP = 32                # strips of 4 basins
NBLOCK = 8                 # blocks of 16 basins (4 strips)
GB = 16
HID = 64
CH = 512                   # time chunk (PSUM bank width in fp32)
NCHUNK = T // CH
KS = 96                    # 4 basins x 24 stacked features


def _split_multiwaits(nc):
    """This container's walrus accepts only ONE sem wait per instruction
    ("Too many sync wait commands").  Hoist surplus waits onto single-wait
    NoOps inserted immediately before the instruction on the same engine
    queue (engines execute block-order, so gating is preserved)."""
    uid = [0]
    for fn in nc.m.functions:
        for bb in fn.blocks:
            new = []
            for inst in bb.instructions:
                si = inst.sync_info
                waits = list(si.on_wait) if si is not None and si.on_wait else []
                if len(waits) > 1:
                    for w in waits[:-1]:
                        uid[0] += 1
                        new.append(
                            mybir.InstNoOp(
                                name=f"{inst.name}-sw{uid[0]}",
                                engine=inst.engine,
                                bass_nofuse=True,
                                sync_info=mybir.SyncInfo(on_wait=[w], on_update=[]),
                            )
                        )
                    si.on_wait = waits[-1:]
                new.append(inst)
            bb.instructions = new


def build_nc(n_chunks=NCHUNK, n_blocks=NBLOCK, split_waits=True):
    nc = bass.Bass()
    xp = nc.declare_dram_parameter("xp", [NSTRIP, KS, T], F32R, isOutput=False)
    l1 = nc.declare_dram_parameter("l1", [2, KS, 128], F32R, isOutput=False)
    l2 = nc.declare_dram_parameter("l2", [128, 128], F32R, isOutput=False)
    l3 = nc.declare_dram_parameter("l3", [8, 128, GB], F32R, isOutput=False)
    b1 = nc.declare_dram_parameter("b1", [128, 1], F32, isOutput=False)
    b2 = nc.declare_dram_parameter("b2", [128, 1], F32, isOutput=False)
    bo = nc.declare_dram_parameter("bo", [GB, 1], F32, isOutput=False)
    y = nc.declare_dram_parameter("y", [NBLOCK, GB, T], F32, isOutput=True)

    with TileContext(nc) as tc:
        with (
            tc.tile_pool(name="const", bufs=1) as constp,
            tc.tile_pool(name="xs", bufs=6) as xsp,
            tc.tile_pool(name="act", bufs=4) as actp,
            tc.tile_pool(name="yout", bufs=2) as youtp,
            tc.tile_pool(name="psumh", bufs=2, space=bass.MemorySpace.PSUM) as psumhp,
            tc.tile_pool(name="psuma", bufs=2, space=bass.MemorySpace.PSUM) as psumap,
            tc.tile_pool(name="psumy", bufs=2, space=bass.MemorySpace.PSUM) as psumyp,
        ):
            # ---- constants (loaded once) ----
            l1t = {}
            for q in range(2):
                tl = constp.tile([KS, 128], F32R, name=f"l1_{q}")
                nc.sync.dma_start(out=tl, in_=l1[q])
                l1t[q] = tl
            l2t = constp.tile([128, 128], F32R, name="l2t")
            nc.sync.dma_start(out=l2t, in_=l2[:])
            l3t = {}
            for p in range(8):
                tl = constp.tile([128, GB], F32R, name=f"l3_{p}")
                nc.sync.dma_start(out=tl, in_=l3[p])
                l3t[p] = tl
            b1t = constp.tile([128, 1], F32, name="b1t")
            nc.sync.dma_start(out=b1t, in_=b1[:])
            b2t = constp.tile([128, 1], F32, name="b2t")
            nc.sync.dma_start(out=b2t, in_=b2[:])
            bot = constp.tile([GB, 1], F32, name="bot")
            nc.sync.dma_start(out=bot, in_=bo[:])

            # ---- main loops ----
            for ck in range(n_chunks):
                t0 = ck * CH
                for J in range(n_blocks):
                    psY = psumyp.tile([GB, CH], F32, name="psY", tag="psY")
                    Atiles = []
                    for sg in range(4):  # strips within the block
                        strip = 4 * J + sg
                        xs = xsp.tile([KS, CH], F32R, name="xs", tag="xs")
                        nc.sync.dma_start(out=xs, in_=xp[strip][:, t0 : t0 + CH])

                        psH = psumhp.tile([128, 2 * CH], F32, name="psH", tag="psH")
                        for q in range(2):
                            nc.tensor.matmul(
                                psH[:, CH * q : CH * q + CH],
                                l1t[q],
                                xs,
                                start=True,
                                stop=True,
                            )
                        H = actp.tile([128, 2 * CH], F32R, name="H", tag="H")
                        # relu on DVE (frees the scalar engine for tanh)
                        nc.vector.tensor_scalar(
                            H, psH, b1t, 0.0, mybir.AluOpType.add, mybir.AluOpType.max
                        )
                        for q in range(2):
                            psA = psumap.tile([128, CH], F32, name="psA", tag="psA")
                            nc.tensor.matmul(
                                psA,
                                l2t,
                                H[:, CH * q : CH * q + CH],
                                start=True,
                                stop=True,
                            )
                            A = actp.tile([128, CH], F32R, name="A", tag="A", bufs=10)
                            nc.scalar.activation(A, psA, AF.Tanh, bias=b2t)
                            Atiles.append(A)
                    # layer-3 accumulation chain deferred to the block tail so
                    # tanh latency can't head-of-line-block the PE queue
                    for p, A in enumerate(Atiles):
                        nc.tensor.matmul(
                            psY,
                            l3t[p],
                            A,
                            start=(p == 0),
                            stop=(p == 7),
                        )

                    ysb = youtp.tile([GB, CH], F32, name="ysb", tag="ysb")
                    nc.vector.tensor_scalar_add(ysb, psY, bot)
                    nc.sync.dma_start(out=y[J][:, t0 : t0 + CH], in_=ysb)
    if split_waits:
        _split_multiwaits(nc)
    return nc


def prep_weights(W_in, b_in, W_ih, b_ih, b_hh, W_out, b_out):
    W_in = np.asarray(W_in, np.float32)
    # fold the 32 reference features (duplicated t-1 slice) into 24:
    # A_d [64, 8] acts on all 8 channels of x[t-d]
    A = np.zeros((3, HID, 8), np.float32)  # A[d-1]
    A[0, :, 0:7] = W_in[:, 0:7] + W_in[:, 21:28]   # t-1, channels 0..6
    A[0, :, 7] = W_in[:, 28] + W_in[:, 31]         # t-1, channel 7 (y)
    A[1, :, 0:7] = W_in[:, 14:21]                  # t-2
    A[1, :, 7] = W_in[:, 30]
    A[2, :, 0:7] = W_in[:, 7:14]                   # t-3
    A[2, :, 7] = W_in[:, 29]

    # l1[q][24*b + 8*(d-1) + c, 64*s + j] = A_d[j, c] for b = 2q + s
    l1 = np.zeros((2, KS, 128), np.float32)
    for q in range(2):
        for s in range(2):
            b = 2 * q + s
            for d in range(3):
                l1[q, 24 * b + 8 * d : 24 * b + 8 * d + 8, 64 * s : 64 * s + 64] = A[d].T

    l2 = np.zeros((128, 128), np.float32)
    l2[0:64, 0:64] = np.asarray(W_ih, np.float32).T
    l2[64:128, 64:128] = np.asarray(W_ih, np.float32).T

    W_out = np.asarray(W_out, np.float32)  # [1, 64]
    l3 = np.zeros((8, 128, GB), np.float32)
    for p in range(8):
        l3[p, 0:64, 2 * p] = W_out[0]
        l3[p, 64:128, 2 * p + 1] = W_out[0]

    b1 = np.concatenate([b_in, b_in]).astype(np.float32).reshape(128, 1)
    bb = np.asarray(b_ih, np.float32) + np.asarray(b_hh, np.float32)
    b2 = np.concatenate([bb, bb]).astype(np.float32).reshape(128, 1)
    bo = np.full((GB, 1), np.float32(np.asarray(b_out, np.float32)[0]))
    return l1, l2, l3, b1, b2, bo


def prep_x_core(x, core):
    """x [4096, 1024, 8] -> xp [32, 96, 4096] delay-stacked channel-major."""
    xc = np.asarray(x[:, core * G_CORE : (core + 1) * G_CORE, :], np.float32)
    xcm = np.ascontiguousarray(xc.transpose(1, 2, 0))  # [g, c, t]
    out = np.zeros((NSTRIP, 4, 3, 8, T), np.float32)
    for d in (1, 2, 3):
        out[:, :, d - 1, :, d:] = xcm[:, :, : T - d].reshape(NSTRIP, 4, 8, T - d)
    return out.reshape(NSTRIP, KS, T)


_NC_CACHE = {}


def _get_nc():
    if "nc" not in _NC_CACHE:
        _NC_CACHE["nc"] = build_nc()
    return _NC_CACHE["nc"]


def kernel(x, W_in, b_in, W_ih, b_ih, W_hh, b_hh, W_out, b_out, _trace=False):
    from concourse.bass_utils import run_bass_kernel_spmd

    x = np.asarray(x, np.float32)
    l1, l2, l3, b1, b2, bo = prep_weights(W_in, b_in, W_ih, b_ih, b_hh, W_out, b_out)
    in_maps = []
    for core in range(NCORES):
        in_maps.append(
            {
                "xp": prep_x_core(x, core),
                "l1": l1,
                "l2": l2,
                "l3": l3,
                "b1": b1,
                "b2": b2,
                "bo": bo,
            }
        )
    nc = _get_nc()
    res = run_bass_kernel_spmd(nc, in_maps, list(range(NCORES)), trace=_trace)
    _NC_CACHE["last_result"] = res

    out = np.empty((T, NG_ALL, 1), np.float32)
    out[:3, :, 0] = x[:3, :, 7]
    for core in range(NCORES):
        yc = res.results[core]["y"].reshape(G_CORE, T)  # [g_local, t]
        out[3:, core * G_CORE : (core + 1) * G_CORE, 0] = yc[:, 3:].T
    return out
